# revision 3
# baseline (speedup 1.0000x reference)
"""MetaUpscale (Meta-SR) Trainium2 kernel.

out[b,o,i,j] = sum_{c,ky,kx} xpad[b,c,(i//2)+ky,(j//2)+kx] * w[i*OW+j, (c*3+ky)*3+kx, o]

Shapes: x [4,64,96,96] f32, weight [36864, 576, 3] f32 -> out [4,3,192,192] f32.

Strategy (memory-bound: the 255MB weight tensor dominates; per-core HBM cap
~358 GB/s):
- Shard over output rows: core r handles out rows [24r, 24r+24) i.e. source
  rows a in [12r, 12r+12).
- All streamed tensors are bf16 (host pre-cast): weight shard 15.9MB/core,
  x slabs ~2.6MB.  Error budget (2e-2) tolerates bf16's ~2e-3 comfortably.
- Host pre-transposes weight into a per-partition-contiguous stream of
  per-(group-pair) blocks [128, 9, 384]: partitions 0-63 hold group 2g's
  taps (c=64 rows), partitions 64-127 hold group 2g+1's.  Blocks are DMAed
  in chunks of 1-2 pairs (0.9-1.8MB) on the SP HWDGE ring.
- Per group (32 source patches): 5 accumulating PE matmuls (4x K=128 +
  1x K=64) contracting c x taps.  lhsT [c, (p,b)] is sliced directly from
  the SBUF x slab (layout [c, h, w, b]); the slab is duplicated w+1-shifted
  on partitions 64-127, plus an h+1-shifted copy for the fused
  (ky=0,kx=2)+(ky=1,kx=2) tap.  PSUM [128,384] holds the full outer
  product; columns ordered (di,dj,o,p) so the useful entries of output q
  sit in 32-blocks.
- Extraction: ACT cast-copies PSUM f32 -> SBUF bf16 (frees PSUM early);
  DVE multiplies by a constant 0/1 mask (all-bf16, contiguous -> 2x/4x
  perf mode) and block-reduces 32->1 per q.  Each 32-block has exactly one
  nonzero so the bf16 reduce is exact; results stream out as bf16.
"""

import numpy as np
import ml_dtypes

import concourse.bacc as bacc
import concourse.mybir as mybir
import concourse.tile as tile
from concourse.bass_utils import run_bass_kernel_spmd
import concourse.bass_utils as _bu

B, C, KS = 4, 64, 3
H = W = 96
OH = OW = 192
NCORES = 8
AROWS = 12            # source rows per core
HS, WS = AROWS + 2, W + 2
NP = 32               # source patches (columns) per group
NCOL = NP * 12        # 384 matmul rhs columns per tap
NGRP = AROWS * 3      # 36 groups per core (a_loc x j_grp)
NPAIR = NGRP // 2
PAIR_CHUNKS = [1, 1] + [2] * 8   # wt DMA chunking (pairs per transfer)

_DT = mybir.dt
_BF16 = ml_dtypes.bfloat16


def _build_nc():
    dt_mm = _DT.bfloat16
    nc = bacc.Bacc("TRN2", target_bir_lowering=False, debug=False)
    xs_d = nc.dram_tensor("xs", [128, HS, WS, B], dt_mm, kind="ExternalInput").ap()
    HEAD = 6  # slab rows 0..5 arrive first so pair-0 matmuls start early
    wt_d = nc.dram_tensor("wt", [128, NPAIR, 9, NCOL], dt_mm, kind="ExternalInput").ap()
    xsh_d = nc.dram_tensor("xsh", [128, AROWS, WS, B], dt_mm, kind="ExternalInput").ap()
    mask_d = nc.dram_tensor("mask", [128, NCOL], dt_mm, kind="ExternalInput").ap()
    out_d = nc.dram_tensor("out", [128, NGRP * 12], dt_mm, kind="ExternalOutput").ap()

    with nc.allow_low_precision(reason="masked 32-block reduce has exactly one "
                                "nonzero summand; bf16 out is a single rounding"):
        with tile.TileContext(nc) as tc:
            with (
                tc.tile_pool(name="xs", bufs=1) as xs_pool,
                tc.tile_pool(name="msk", bufs=1) as msk_pool,
                tc.tile_pool(name="res", bufs=3) as res_pool,
                tc.tile_pool(name="wt1", bufs=2) as wt1_pool,
                tc.tile_pool(name="wt2", bufs=4) as wt2_pool,
                tc.tile_pool(name="pcp", bufs=4) as pcp_pool,
                tc.tile_pool(name="tmp", bufs=4) as tmp_pool,
                tc.tile_pool(name="ps", bufs=6, space="PSUM") as ps_pool,
            ):
                # xs heads the SP ring so the first matmul's lhsT arrives ASAP;
                # weight chunks follow on SP; xs tail/mask ride the ACT ring;
                # outputs ride SWDGE (gpsimd).
                xh_t = xs_pool.tile([128, HEAD, WS, B], dt_mm, tag="xh")
                nc.sync.dma_start(xh_t[:], xs_d[:, 0:HEAD])
                xt_t = xs_pool.tile([128, HS - 4, WS, B], dt_mm, tag="xt")
                nc.scalar.dma_start(xt_t[:], xs_d[:, 4:HS])
                msk_t = msk_pool.tile([128, NCOL], dt_mm)
                nc.scalar.dma_start(msk_t[:], mask_d)

                def xslab(h):
                    return (xh_t, h) if h < HEAD else (xt_t, h - 4)

                # T_H slab: partitions 0-63 = xs rows 0..11, partitions 64-127 =
                # xs rows 1..12 (h+1 shift).  Lets the (ky=0,kx=2)+(ky=1,kx=2)
                # taps run as one K=128 matmul at a w-offset of 2.
                thh_t = xs_pool.tile([128, 4, WS, B], dt_mm, tag="thh")
                nc.sync.dma_start(thh_t[:], xsh_d[:, 0:4])
                tht_t = xs_pool.tile([128, 8, WS, B], dt_mm, tag="tht")
                nc.scalar.dma_start(tht_t[:], xsh_d[:, 4:12])

                def thslab(a_loc):
                    return (thh_t, a_loc) if a_loc < 4 else (tht_t, a_loc - 4)

                gp = 0
                for nch in PAIR_CHUNKS:
                    pool = wt1_pool if nch == 1 else wt2_pool
                    wt_t = pool.tile([128, nch, 9, NCOL], dt_mm)
                    nc.sync.dma_start(wt_t[:], wt_d[:, gp:gp + nch])

                    for ci in range(nch):
                        res_t = res_pool.tile([128, 2 * 12], dt_mm, tag="res")
                        for half in range(2):
                            g = 2 * gp + half
                            a_loc, jg = g // 3, g % 3
                            lo = 64 * half
                            ps_t = ps_pool.tile([128, NCOL], _DT.float32)
                            # 3x K=128 matmuls: kx=0 on partitions 0-63 (plain
                            # slab), kx=1 on 64-127 (w+1-shifted slab copy)
                            for ky in range(3):
                                xt_, h = xslab(a_loc + ky)
                                lhsT = xt_[:, h, jg * NP : jg * NP + NP, :]
                                nc.tensor.matmul(
                                    ps_t[:], lhsT, wt_t[:, ci, 3 * half + ky, :],
                                    start=(ky == 0), stop=False,
                                )
                            # K=128 matmul: (ky=0,kx=2) + (ky=1,kx=2) via the
                            # h-shifted T_H slab at w-offset 2
                            th_, ha = thslab(a_loc)
                            lhsT = th_[:, ha, jg * NP + 2 : jg * NP + 2 + NP, :]
                            nc.tensor.matmul(
                                ps_t[:], lhsT, wt_t[:, ci, 6 + half, :],
                                start=False, stop=False,
                            )
                            # K=64 leftover (ky=2, kx=2); the two groups of the
                            # pair use opposite partition halves (odd half's slab
                            # is pre-shifted by one column, hence the offset)
                            xt_, h = xslab(a_loc + 2)
                            off = jg * NP + (2 - half)
                            lhsT = xt_[lo : lo + C, h, off : off + NP, :]
                            nc.tensor.matmul(
                                ps_t[:], lhsT, wt_t[lo : lo + C, ci, 8, :],
                                start=False, stop=True,
                            )

                            # drain PSUM on ACT (f32 -> bf16), then mask+reduce
                            # on DVE in all-bf16 fast mode
                            pcp_t = pcp_pool.tile([128, NCOL], dt_mm)
                            nc.scalar.copy(pcp_t[:], ps_t[:])
                            tmp_t = tmp_pool.tile([128, NCOL], dt_mm)
                            nc.vector.tensor_mul(tmp_t[:], pcp_t[:], msk_t[:])
                            nc.vector.reduce_sum(
                                res_t[:, half * 12 : half * 12 + 12],
                                tmp_t[:].rearrange("p (q k) -> p q k", k=NP),
                                axis=mybir.AxisListType.X,
                            )
                        nc.gpsimd.dma_start(
                            out_d[:, gp * 24 : gp * 24 + 24], res_t[:])
                        gp += 1
    nc.finalize()
    return nc


def _host_prep(x, weight):
    """Returns per-core in_maps for the 8 cores."""
    xpad = np.pad(x, ((0, 0), (0, 0), (1, 1), (1, 1)))
    # [c, h, w, b] so lhsT window columns are contiguous
    xt = np.ascontiguousarray(xpad.transpose(1, 2, 3, 0)).astype(_BF16)

    # weight [OH*OW, 576, 3] -> [a, di, jg, p, dj, c, ky, kx, o]
    w9 = weight.reshape(H, 2, 3, NP, 2, C, KS, KS, 3)
    # -> [a, jg, ky, kx, c, di, dj, o, p]  (columns q=(di,dj,o) outer, p inner)
    wt = np.ascontiguousarray(w9.transpose(0, 2, 6, 7, 5, 1, 4, 8, 3)).astype(_BF16)
    wt = wt.reshape(H, 3, 9, C, NCOL)

    # mask[m, q*32+p] = 1 iff p == m//4
    mask = np.zeros((128, NCOL), dtype=_BF16)
    for m in range(128):
        p = m // B
        mask[m, p::NP] = 1.0

    xt_shift = np.zeros_like(xt)
    xt_shift[:, :, :-1] = xt[:, :, 1:]                  # slab shifted by w+1

    in_maps = []
    for r in range(NCORES):
        sl = slice(12 * r, 12 * r + HS)
        xs2 = np.concatenate([xt[:, sl], xt_shift[:, sl]], axis=0)
        wtr = wt[AROWS * r : AROWS * (r + 1)].reshape(NGRP, 9, C, NCOL)
        wa = wtr[0::2].reshape(NPAIR, 3, 3, C, NCOL)    # pair ky kx c n
        wb = wtr[1::2].reshape(NPAIR, 3, 3, C, NCOL)
        wtp = np.empty((NPAIR, 128, 9, NCOL), _BF16)
        wtp[:, 0:64, 0:3] = wa[:, :, 0].transpose(0, 2, 1, 3)
        wtp[:, 64:128, 0:3] = wa[:, :, 1].transpose(0, 2, 1, 3)
        wtp[:, 0:64, 3:6] = wb[:, :, 0].transpose(0, 2, 1, 3)
        wtp[:, 64:128, 3:6] = wb[:, :, 1].transpose(0, 2, 1, 3)
        wtp[:, 0:64, 6] = wa[:, 0, 2]
        wtp[:, 64:128, 6] = wa[:, 1, 2]
        wtp[:, 0:64, 7] = wb[:, 0, 2]
        wtp[:, 64:128, 7] = wb[:, 1, 2]
        wtp[:, 0:64, 8] = wa[:, 2, 2]
        wtp[:, 64:128, 8] = wb[:, 2, 2]
        # per-partition-contiguous pair stream [128, NPAIR, 9, NCOL]
        wtp = np.ascontiguousarray(wtp.transpose(1, 0, 2, 3))
        xsh = np.concatenate([xt[:, 12 * r : 12 * r + AROWS],
                              xt[:, 12 * r + 1 : 12 * r + 1 + AROWS]], axis=0)
        in_maps.append({"xs": xs2, "xsh": xsh, "wt": wtp, "mask": mask})
    return in_maps


def _host_gather(results):
    """results: list of 8 dicts with 'out' [128, 432] bf16 -> full [B,3,OH,OW]."""
    res = np.stack([r["out"] for r in results]).astype(np.float32)
    res = res.reshape(NCORES, NP, B, AROWS, 3, 2, 2, 3)    # r p b a_loc jg di dj o
    out = res.transpose(2, 7, 0, 3, 5, 4, 1, 6)            # b o r a_loc di jg p dj
    return np.ascontiguousarray(out.reshape(B, 3, OH, OW))


_CACHED_NC = None


def _get_nc():
    global _CACHED_NC
    if _CACHED_NC is None:
        _CACHED_NC = _build_nc()
    return _CACHED_NC


def kernel(x, weight, **run_kwargs):
    x = np.asarray(x, dtype=np.float32)
    weight = np.asarray(weight, dtype=np.float32)
    in_maps = _host_prep(x, weight)
    nc = _get_nc()
    r = run_bass_kernel_spmd(nc, in_maps, core_ids=list(range(NCORES)), **run_kwargs)
    out = _host_gather(r.results)
    kernel.last_result = r
    return out


# revision 6
# speedup vs baseline: 1.0158x; 1.0158x over previous
"""MetaUpscale (Meta-SR) Trainium2 kernel.

out[b,o,i,j] = sum_{c,ky,kx} xpad[b,c,(i//2)+ky,(j//2)+kx] * w[i*OW+j, (c*3+ky)*3+kx, o]

Shapes: x [4,64,96,96] f32, weight [36864, 576, 3] f32 -> out [4,3,192,192] f32.

Strategy (memory-bound: the 255MB weight tensor dominates; per-core HBM cap
~358 GB/s):
- Shard over output rows: core r handles out rows [24r, 24r+24) i.e. source
  rows a in [12r, 12r+12).
- All streamed tensors are bf16 (host pre-cast): weight shard 15.9MB/core,
  x slabs ~2.6MB.  Error budget (2e-2) tolerates bf16's ~2e-3 comfortably.
- Host pre-transposes weight into a per-partition-contiguous stream of
  per-(group-pair) blocks [128, 9, 384]: partitions 0-63 hold group 2g's
  taps (c=64 rows), partitions 64-127 hold group 2g+1's.  Blocks are DMAed
  in chunks of 1-2 pairs (0.9-1.8MB) on the SP HWDGE ring.
- Per group (32 source patches): 5 accumulating PE matmuls (4x K=128 +
  1x K=64) contracting c x taps.  lhsT [c, (p,b)] is sliced directly from
  the SBUF x slab (layout [c, h, w, b]); the slab is duplicated w+1-shifted
  on partitions 64-127, plus an h+1-shifted copy for the fused
  (ky=0,kx=2)+(ky=1,kx=2) tap.  PSUM [128,384] holds the full outer
  product; columns ordered (di,dj,o,p) so the useful entries of output q
  sit in 32-blocks.
- Extraction: ACT cast-copies PSUM f32 -> SBUF bf16 (frees PSUM early);
  DVE multiplies by a constant 0/1 mask (all-bf16, contiguous -> 2x/4x
  perf mode) and block-reduces 32->1 per q.  Each 32-block has exactly one
  nonzero so the bf16 reduce is exact; results stream out as bf16.
"""

import numpy as np
import ml_dtypes

import concourse.bacc as bacc
import concourse.mybir as mybir
import concourse.tile as tile
from concourse.bass_utils import run_bass_kernel_spmd
import concourse.bass_utils as _bu

B, C, KS = 4, 64, 3
H = W = 96
OH = OW = 192
NCORES = 8
AROWS = 12            # source rows per core
HS, WS = AROWS + 2, W + 2
NP = 32               # source patches (columns) per group
NCOL = NP * 12        # 384 matmul rhs columns per tap
NGRP = AROWS * 3      # 36 groups per core (a_loc x j_grp)
NPAIR = NGRP // 2
# wt DMA chunking (pairs per transfer): fine at the start (prompt first
# matmuls) and at the end (shorter drain tail), coarse in the middle
PAIR_CHUNKS = [1, 1, 1, 1, 2, 2, 2, 2, 2, 2, 1, 1]

_DT = mybir.dt
_BF16 = ml_dtypes.bfloat16


def _build_nc():
    dt_mm = _DT.bfloat16
    nc = bacc.Bacc("TRN2", target_bir_lowering=False, debug=False)
    xs_d = nc.dram_tensor("xs", [128, HS, WS, B], dt_mm, kind="ExternalInput").ap()
    HEAD = 6  # slab rows 0..5 arrive first so pair-0 matmuls start early
    wt_d = nc.dram_tensor("wt", [128, NPAIR, 9, NCOL], dt_mm, kind="ExternalInput").ap()
    xsh_d = nc.dram_tensor("xsh", [128, AROWS, WS, B], dt_mm, kind="ExternalInput").ap()
    mask_d = nc.dram_tensor("mask", [128, NCOL], dt_mm, kind="ExternalInput").ap()
    out_d = nc.dram_tensor("out", [128, NGRP * 12], dt_mm, kind="ExternalOutput").ap()

    with nc.allow_low_precision(reason="masked 32-block reduce has exactly one "
                                "nonzero summand; bf16 out is a single rounding"):
        with tile.TileContext(nc) as tc:
            with (
                tc.tile_pool(name="xs", bufs=1) as xs_pool,
                tc.tile_pool(name="msk", bufs=1) as msk_pool,
                tc.tile_pool(name="res", bufs=3) as res_pool,
                tc.tile_pool(name="wt1", bufs=2) as wt1_pool,
                tc.tile_pool(name="wt2", bufs=4) as wt2_pool,
                tc.tile_pool(name="pcp", bufs=4) as pcp_pool,
                tc.tile_pool(name="tmp", bufs=4) as tmp_pool,
                tc.tile_pool(name="ps", bufs=6, space="PSUM") as ps_pool,
            ):
                # Startup is weight-starved: the first pairs need only xs rows
                # 0-2 + T_H row 0 + wt pair 0, so exactly those head the SP
                # ring, interleaved with the first wt chunks in need-order.
                # The x tails (rows 4+, needed from pair ~6) are issued from
                # the ACT queue AFTER pair 1's extraction so their bytes don't
                # steal SDMA bandwidth from the critical early weight chunks.
                # Outputs ride SWDGE (gpsimd).
                xh0_t = xs_pool.tile([128, 3, WS, B], dt_mm, tag="xh0")
                nc.sync.dma_start(xh0_t[:], xs_d[:, 0:3])
                thh0_t = xs_pool.tile([128, 1, WS, B], dt_mm, tag="thh0")
                nc.sync.dma_start(thh0_t[:], xsh_d[:, 0:1])
                msk_t = msk_pool.tile([128, NCOL], dt_mm)
                nc.scalar.dma_start(msk_t[:], mask_d)

                xh1_t = xs_pool.tile([128, 3, WS, B], dt_mm, tag="xh1")
                thh1_t = xs_pool.tile([128, 3, WS, B], dt_mm, tag="thh1")
                xt_t = xs_pool.tile([128, HS - 6, WS, B], dt_mm, tag="xt")
                tht_t = xs_pool.tile([128, 8, WS, B], dt_mm, tag="tht")

                def xslab(h):
                    if h < 3:
                        return (xh0_t, h)
                    if h < 6:
                        return (xh1_t, h - 3)
                    return (xt_t, h - 6)

                # T_H slab: partitions 0-63 = xs rows 0..11, partitions 64-127 =
                # xs rows 1..12 (h+1 shift).  Lets the (ky=0,kx=2)+(ky=1,kx=2)
                # taps run as one K=128 matmul at a w-offset of 2.
                def thslab(a_loc):
                    if a_loc < 1:
                        return (thh0_t, a_loc)
                    if a_loc < 4:
                        return (thh1_t, a_loc - 1)
                    return (tht_t, a_loc - 4)

                gp = 0
                for ich, nch in enumerate(PAIR_CHUNKS):
                    pool = wt1_pool if nch == 1 else wt2_pool
                    wt_t = pool.tile([128, nch, 9, NCOL], dt_mm)
                    nc.sync.dma_start(wt_t[:], wt_d[:, gp:gp + nch])
                    if ich == 0:
                        # rows 3-5 + T_H rows 1-3: needed from pair 1 on
                        nc.sync.dma_start(xh1_t[:], xs_d[:, 3:6])
                        nc.sync.dma_start(thh1_t[:], xsh_d[:, 1:4])
                    elif ich == 2:
                        # x tails (needed from pair ~6): issued on the ACT
                        # queue so they launch only after pair-1 extraction,
                        # clear of the critical early weight chunks
                        nc.scalar.dma_start(xt_t[:], xs_d[:, 6:HS])
                        nc.scalar.dma_start(tht_t[:], xsh_d[:, 4:12])

                    for ci in range(nch):
                        res_t = res_pool.tile([128, 2 * 12], dt_mm, tag="res")
                        for half in range(2):
                            g = 2 * gp + half
                            a_loc, jg = g // 3, g % 3
                            lo = 64 * half
                            ps_t = ps_pool.tile([128, NCOL], _DT.float32)
                            # 3x K=128 matmuls: kx=0 on partitions 0-63 (plain
                            # slab), kx=1 on 64-127 (w+1-shifted slab copy)
                            for ky in range(3):
                                xt_, h = xslab(a_loc + ky)
                                lhsT = xt_[:, h, jg * NP : jg * NP + NP, :]
                                nc.tensor.matmul(
                                    ps_t[:], lhsT, wt_t[:, ci, 3 * half + ky, :],
                                    start=(ky == 0), stop=False,
                                )
                            # K=128 matmul: (ky=0,kx=2) + (ky=1,kx=2) via the
                            # h-shifted T_H slab at w-offset 2
                            th_, ha = thslab(a_loc)
                            lhsT = th_[:, ha, jg * NP + 2 : jg * NP + 2 + NP, :]
                            nc.tensor.matmul(
                                ps_t[:], lhsT, wt_t[:, ci, 6 + half, :],
                                start=False, stop=False,
                            )
                            # K=64 leftover (ky=2, kx=2); the two groups of the
                            # pair use opposite partition halves (odd half's slab
                            # is pre-shifted by one column, hence the offset)
                            xt_, h = xslab(a_loc + 2)
                            off = jg * NP + (2 - half)
                            lhsT = xt_[lo : lo + C, h, off : off + NP, :]
                            nc.tensor.matmul(
                                ps_t[:], lhsT, wt_t[lo : lo + C, ci, 8, :],
                                start=False, stop=True,
                            )

                            # drain PSUM on ACT (f32 -> bf16), then mask+reduce
                            # on DVE in all-bf16 fast mode
                            pcp_t = pcp_pool.tile([128, NCOL], dt_mm)
                            nc.scalar.copy(pcp_t[:], ps_t[:])
                            tmp_t = tmp_pool.tile([128, NCOL], dt_mm)
                            nc.vector.tensor_mul(tmp_t[:], pcp_t[:], msk_t[:])
                            nc.vector.reduce_sum(
                                res_t[:, half * 12 : half * 12 + 12],
                                tmp_t[:].rearrange("p (q k) -> p q k", k=NP),
                                axis=mybir.AxisListType.X,
                            )
                        nc.gpsimd.dma_start(
                            out_d[:, gp * 24 : gp * 24 + 24], res_t[:])
                        gp += 1
    nc.finalize()
    return nc


def _host_prep(x, weight):
    """Returns per-core in_maps for the 8 cores."""
    xpad = np.pad(x, ((0, 0), (0, 0), (1, 1), (1, 1)))
    # [c, h, w, b] so lhsT window columns are contiguous
    xt = np.ascontiguousarray(xpad.transpose(1, 2, 3, 0)).astype(_BF16)

    # weight [OH*OW, 576, 3] -> [a, di, jg, p, dj, c, ky, kx, o]
    w9 = weight.reshape(H, 2, 3, NP, 2, C, KS, KS, 3)
    # -> [a, jg, ky, kx, c, di, dj, o, p]  (columns q=(di,dj,o) outer, p inner)
    wt = np.ascontiguousarray(w9.transpose(0, 2, 6, 7, 5, 1, 4, 8, 3)).astype(_BF16)
    wt = wt.reshape(H, 3, 9, C, NCOL)

    # mask[m, q*32+p] = 1 iff p == m//4
    mask = np.zeros((128, NCOL), dtype=_BF16)
    for m in range(128):
        p = m // B
        mask[m, p::NP] = 1.0

    xt_shift = np.zeros_like(xt)
    xt_shift[:, :, :-1] = xt[:, :, 1:]                  # slab shifted by w+1

    in_maps = []
    for r in range(NCORES):
        sl = slice(12 * r, 12 * r + HS)
        xs2 = np.concatenate([xt[:, sl], xt_shift[:, sl]], axis=0)
        wtr = wt[AROWS * r : AROWS * (r + 1)].reshape(NGRP, 9, C, NCOL)
        wa = wtr[0::2].reshape(NPAIR, 3, 3, C, NCOL)    # pair ky kx c n
        wb = wtr[1::2].reshape(NPAIR, 3, 3, C, NCOL)
        wtp = np.empty((NPAIR, 128, 9, NCOL), _BF16)
        wtp[:, 0:64, 0:3] = wa[:, :, 0].transpose(0, 2, 1, 3)
        wtp[:, 64:128, 0:3] = wa[:, :, 1].transpose(0, 2, 1, 3)
        wtp[:, 0:64, 3:6] = wb[:, :, 0].transpose(0, 2, 1, 3)
        wtp[:, 64:128, 3:6] = wb[:, :, 1].transpose(0, 2, 1, 3)
        wtp[:, 0:64, 6] = wa[:, 0, 2]
        wtp[:, 64:128, 6] = wa[:, 1, 2]
        wtp[:, 0:64, 7] = wb[:, 0, 2]
        wtp[:, 64:128, 7] = wb[:, 1, 2]
        wtp[:, 0:64, 8] = wa[:, 2, 2]
        wtp[:, 64:128, 8] = wb[:, 2, 2]
        # per-partition-contiguous pair stream [128, NPAIR, 9, NCOL]
        wtp = np.ascontiguousarray(wtp.transpose(1, 0, 2, 3))
        xsh = np.concatenate([xt[:, 12 * r : 12 * r + AROWS],
                              xt[:, 12 * r + 1 : 12 * r + 1 + AROWS]], axis=0)
        in_maps.append({"xs": xs2, "xsh": xsh, "wt": wtp, "mask": mask})
    return in_maps


def _host_gather(results):
    """results: list of 8 dicts with 'out' [128, 432] bf16 -> full [B,3,OH,OW]."""
    res = np.stack([r["out"] for r in results]).astype(np.float32)
    res = res.reshape(NCORES, NP, B, AROWS, 3, 2, 2, 3)    # r p b a_loc jg di dj o
    out = res.transpose(2, 7, 0, 3, 5, 4, 1, 6)            # b o r a_loc di jg p dj
    return np.ascontiguousarray(out.reshape(B, 3, OH, OW))


_CACHED_NC = None


def _get_nc():
    global _CACHED_NC
    if _CACHED_NC is None:
        _CACHED_NC = _build_nc()
    return _CACHED_NC


def kernel(x, weight, **run_kwargs):
    x = np.asarray(x, dtype=np.float32)
    weight = np.asarray(weight, dtype=np.float32)
    in_maps = _host_prep(x, weight)
    nc = _get_nc()
    r = run_bass_kernel_spmd(nc, in_maps, core_ids=list(range(NCORES)), **run_kwargs)
    out = _host_gather(r.results)
    kernel.last_result = r
    return out
